# revision 1
# baseline (speedup 1.0000x reference)
"""Strided (residue-group) attention for Trainium2, SPMD across 8 NeuronCores.

Problem: x[B=2,S=4096,E=1024] -> qkv proj -> per-(batch,head,residue-group)
attention (stride 8 -> 8 groups of n=512 tokens) -> out proj.

Sharding: by (batch, residue-group).  B*stride = 16 group-instances; each of
the 8 cores owns 2 (batch,group) pairs = 1024 tokens and computes their FULL
output rows (it holds all 16 heads for its tokens).  The residue groups are
independent, so there are no cross-device collectives at all; the host
permutes tokens into group-major order on the way in and inverts on the way
out.

Device kernel design (per core):
  - Host pre-transposes x so the kernel receives xT [E, 1024tok] (contiguous
    DMA; tensor engine needs the contraction dim on partitions).
  - QKV: qT,kT produced feature-on-partition ([f,tok]); v produced
    token-on-partition ([tok,f]).  All matmuls in float32r (full-rate fp32).
  - scoresT[k,q] = kT.T-chunks @ qT per head; head pairs are row-packed on
    the PE array (K=64 each at array rows 0-63 / 64-127).
  - exp on ScalarE without max-subtraction (scores are O(+-8), exp is safe).
  - PV: lhsT = [v | ones] (even heads) or [ones | v] (odd heads) so one
    accumulation chain yields both o-rows and 64 replicated softmax
    denominator rows; GpSimd partition_broadcast moves the reciprocal row
    across the partition halves, DVE does recip + normalize.
  - out proj: lhsT = oT chunks, rhs = Wout rows -> natural [tok, E] output.
"""

import os

import numpy as np

B, S, E = 2, 4096, 1024
H, ST = 16, 8
DH = E // H  # 64
N = S // ST  # 512 tokens per residue group
NCORES = 8
GPC = (B * ST) // NCORES  # 2 (batch,group) pairs per core
TOK = GPC * N  # 1024 tokens per core
P = 128
EC = E // P  # 8 contraction chunks of 128
NB = N // P  # 4 token chunks of 128 per group
FB = 2  # feature blocks of 512 in E
SCALE = 1.0 / float(np.sqrt(DH))

_CACHE: dict = {}


def _build_nc():
    import concourse.bass as bass
    import concourse.bacc as bacc
    import concourse.tile as tile
    from concourse import mybir

    F32 = mybir.dt.float32
    FP16 = mybir.dt.float16
    ADD = mybir.AluOpType.add
    EXP = mybir.ActivationFunctionType.Exp
    LOG = mybir.ActivationFunctionType.Ln

    nc = bacc.Bacc()
    xt = nc.declare_dram_parameter("xt", [E, TOK], FP16, isOutput=False)
    wq = nc.declare_dram_parameter("wq", [EC, P, EC, P], FP16, isOutput=False)
    wk = nc.declare_dram_parameter("wk", [EC, P, EC, P], FP16, isOutput=False)
    wv = nc.declare_dram_parameter("wv", [E, E], FP16, isOutput=False)
    wo = nc.declare_dram_parameter("wo", [E, E], FP16, isOutput=False)
    bq = nc.declare_dram_parameter("bq", [E], F32, isOutput=False)
    bk = nc.declare_dram_parameter("bk", [E], F32, isOutput=False)
    bv = nc.declare_dram_parameter("bv", [E], F32, isOutput=False)
    bo = nc.declare_dram_parameter("bo", [E], F32, isOutput=False)
    vones = nc.declare_dram_parameter("vones", [H * P], FP16, isOutput=False)
    out = nc.declare_dram_parameter("out", [TOK, E], F32, isOutput=True)

    with tile.TileContext(nc) as tc, (
        tc.tile_pool(name="const", bufs=1)
    ) as const, tc.tile_pool(name="xtp", bufs=17) as xtp, tc.tile_pool(
        name="wqkp", bufs=4
    ) as wqkp, tc.tile_pool(name="wvp", bufs=10) as wvp, tc.tile_pool(
        name="qtp", bufs=9
    ) as qtp, tc.tile_pool(name="ktp", bufs=9) as ktp, tc.tile_pool(
        name="vpp", bufs=5
    ) as vpp, tc.tile_pool(name="expp", bufs=3) as expp, tc.tile_pool(
        name="otp", bufs=17
    ) as otp, tc.tile_pool(name="recp", bufs=4) as recp, tc.tile_pool(
        name="outp", bufs=3
    ) as outp, tc.tile_pool(name="osbp", bufs=18) as osbp, tc.tile_pool(
        name="psmm", bufs=2, space="PSUM"
    ) as psmm, tc.tile_pool(
        name="pssc", bufs=2, space="PSUM"
    ) as pssc, tc.tile_pool(name="pso", bufs=2, space="PSUM") as psop:
        # ---- constants -------------------------------------------------
        bq_sb = const.tile([P, EC], F32)
        nc.sync.dma_start(out=bq_sb, in_=bq[:].rearrange("(c p) -> p c", p=P))
        bk_sb = const.tile([P, EC], F32)
        nc.sync.dma_start(out=bk_sb, in_=bk[:].rearrange("(c p) -> p c", p=P))
        bv_bc = const.tile([P, E], F32)
        nc.gpsimd.dma_start(out=bv_bc, in_=bv[:].partition_broadcast(P))
        bo_bc = const.tile([P, E], F32)
        nc.gpsimd.dma_start(out=bo_bc, in_=bo[:].partition_broadcast(P))
        # Wout resident (fp16): [p, fb, dc, 512]; loaded later, off the
        # startup critical path
        wo_sb = const.tile([P, FB, EC, 512], FP16)

        def load_wo():
            for fb in range(FB):
                nc.sync.dma_start(
                    out=wo_sb[:, fb],
                    in_=wo[:, fb * 512 : (fb + 1) * 512].rearrange(
                        "(c p) f -> p c f", p=P
                    ),
                )

        xt_c = {0: [], 1: []}
        osbs = {0: {}, 1: {}}
        last_evac = {}
        last_recip_act = {}
        first_attn_act = {0: [], 1: []}
        qts = {0: [], 1: []}
        kts = {0: [], 1: []}
        vts = {0: [], 1: []}
        ots = {0: [], 1: []}

        def load_xt(g):
            for c in range(EC):
                t = xtp.tile([P, N], FP16, tag="xt")
                nc.sync.dma_start(
                    out=t,
                    in_=xt[c * P : (c + 1) * P, g * N : (g + 1) * N],
                )
                xt_c[g].append(t)

        def emit_qk_ftile(g, which, ft):
            wmat, bias_sb, lst = (
                (wq, bq_sb, qts[g]) if which == "q" else (wk, bk_sb, kts[g])
            )
            wt = wqkp.tile([P, EC, P], FP16, tag="wqk")
            nc.sync.dma_start(out=wt, in_=wmat[ft])
            ps = psmm.tile([P, N], F32, tag="mm")
            for c in range(EC):
                nc.tensor.matmul(
                    ps,
                    lhsT=wt[:, c, :],
                    rhs=xt_c[g][c],
                    start=(c == 0),
                    stop=(c == EC - 1),
                )
            if which == "q":
                t = qtp.tile([P, N], FP16, tag="qt")
            else:
                t = ktp.tile([P, N], FP16, tag="kt")
            nc.vector.tensor_scalar(
                out=t, in0=ps, scalar1=bias_sb[:, ft : ft + 1], scalar2=None, op0=ADD
            )
            lst.append(t)

        def emit_v_fb(g, fb):
            if fb == 0:
                for tt in range(NB):
                    vt = vpp.tile([P, H, P], FP16, tag="vp")
                    # ones pattern (even heads: cols 64-127; odd: cols 0-63);
                    # the v-projection copies then overwrite the v halves
                    nc.gpsimd.dma_start(
                        out=vt,
                        in_=vones[:]
                        .rearrange("(h d) -> h d", h=H)
                        .partition_broadcast(P),
                    )
                    vts[g].append(vt)
            wv_ts = []
            for c in range(EC):
                wvt = wvp.tile([P, 512], FP16, tag="wv")
                nc.sync.dma_start(
                    out=wvt, in_=wv[c * P : (c + 1) * P, fb * 512 : (fb + 1) * 512]
                )
                wv_ts.append(wvt)
            for tt in range(NB):
                ps = psmm.tile([P, 512], F32, tag="mm")
                for c in range(EC):
                    nc.tensor.matmul(
                        ps,
                        lhsT=xt_c[g][c][:, tt * P : (tt + 1) * P],
                        rhs=wv_ts[c],
                        start=(c == 0),
                        stop=(c == EC - 1),
                    )
                for hl in range(8):
                    h = fb * 8 + hl
                    off = 0 if (h % 2 == 0) else DH
                    nc.vector.tensor_add(
                        out=vts[g][tt][:, h, off : off + DH],
                        in0=ps[:, hl * DH : (hl + 1) * DH],
                        in1=bv_bc[:, fb * 512 + hl * DH : fb * 512 + (hl + 1) * DH],
                    )

        def emit_attn_pair(g, pr):
            # scores for both heads of the pair, row-packed on the PE array
            # (K=64 each at array rows 0-63 / 64-127, separate PSUM banks)
            ex_AB = {}
            for h in (2 * pr, 2 * pr + 1):
                ex_AB[h] = expp.tile([P, NB, N], FP16, tag="exp", name=f"ex{h}")
            for half in range(2):
                scs = {}
                for h in (2 * pr, 2 * pr + 1):
                    lo, hi = (0, DH) if h % 2 == 0 else (DH, P)
                    sc = pssc.tile([P, 2, N], F32, tag="sc")
                    for cc in range(2):
                        c = 2 * half + cc
                        nc.tensor.matmul(
                            sc[:, cc],
                            lhsT=kts[g][pr][lo:hi, c * P : (c + 1) * P],
                            rhs=qts[g][pr][lo:hi, :],
                            start=True,
                            stop=True,
                        )
                    scs[h] = sc
                for h in (2 * pr, 2 * pr + 1):
                    act_i = nc.scalar.activation(
                        out=ex_AB[h][:, 2 * half : 2 * half + 2],
                        in_=scs[h],
                        func=EXP,
                    )
                    if g == 1 and 0 in last_recip_act:
                        # keep group-1 softmax exps after the batch-0 recips
                        # so the ACT table set isn't thrashed
                        tile.add_dep_helper(
                            act_i.ins, last_recip_act[0].ins,
                            reason="phase order: attn(1) exps after recip batch 0",
                        )
            for h in (2 * pr, 2 * pr + 1):
                ex = ex_AB[h]
                po = psop.tile([P, N], F32, tag="po")
                for c in range(NB):
                    nc.tensor.matmul(
                        po,
                        lhsT=vts[g][c][:, h, :],
                        rhs=ex[:, c, :],
                        start=(c == 0),
                        stop=(c == NB - 1),
                    )
                # evacuate PSUM immediately (unnormalized o + denominator
                # rows); normalization happens in a batched phase so the ACT
                # ln/exp calls don't thrash activation-table loads against
                # the softmax exps
                osb = osbp.tile([P, N], FP16, tag="osb")
                last_evac[g] = nc.vector.tensor_copy(out=osb, in_=po)
                osbs[g][h] = osb

        def emit_recip_pair(g, pr):
            ot = otp.tile([P, N], FP16, tag="ot")
            for h in (2 * pr, 2 * pr + 1):
                osb = osbs[g][h]
                rec = recp.tile([P, N], F32, tag="rec")
                rec2 = recp.tile([P, N], F32, tag="rec2")
                if h % 2 == 0:
                    # o rows 0-63; replicated denominator rows 64-127.
                    # 1/denom = exp(-ln d) on ACT (DVE reciprocal is 8x slow);
                    # a replication DMA moves it across the partition halves
                    # (engines cannot cross partitions).
                    ln_i = nc.scalar.activation(
                        out=rec[DH : DH + 1, :], in_=osb[DH : DH + 1, :], func=LOG
                    )
                    tile.add_dep_helper(
                        ln_i.ins, last_evac[g].ins,
                        reason="phase order: recip batch after all evacuations",
                    )
                    last_recip_act[g] = nc.scalar.activation(
                        out=rec[DH : DH + 1, :],
                        in_=rec[DH : DH + 1, :],
                        func=EXP,
                        scale=-1.0,
                    )
                    s = rec[DH : DH + 1, :]
                    nc.sync.dma_start(
                        out=rec2[0:DH, :],
                        in_=bass.AP(
                            tensor=s.tensor,
                            offset=s.offset,
                            ap=[list(s.ap[0]), [0, DH], list(s.ap[1])],
                        ),
                    )
                    nc.vector.tensor_mul(
                        out=ot[0:DH, :], in0=osb[0:DH, :], in1=rec2[0:DH, :]
                    )
                else:
                    # denominator rows 0-63, o rows 64-127
                    ln_i = nc.scalar.activation(out=rec[0:1, :], in_=osb[0:1, :], func=LOG)
                    tile.add_dep_helper(
                        ln_i.ins, last_evac[g].ins,
                        reason="phase order: recip batch after all evacuations",
                    )
                    last_recip_act[g] = nc.scalar.activation(
                        out=rec[0:1, :], in_=rec[0:1, :], func=EXP, scale=-1.0
                    )
                    s = rec[0:1, :]
                    nc.sync.dma_start(
                        out=rec2[DH:P, :],
                        in_=bass.AP(
                            tensor=s.tensor,
                            offset=s.offset,
                            ap=[list(s.ap[0]), [0, DH], list(s.ap[1])],
                        ),
                    )
                    nc.vector.tensor_mul(
                        out=ot[DH:P, :], in0=osb[DH:P, :], in1=rec2[DH:P, :]
                    )
            ots[g].append(ot)

        def emit_outproj_unit(g, fb, tt):
            ps = psmm.tile([P, 512], F32, tag="mm")
            for dc in range(EC):
                nc.tensor.matmul(
                    ps,
                    lhsT=ots[g][dc][:, tt * P : (tt + 1) * P],
                    rhs=wo_sb[:, fb, dc, :],
                    start=(dc == 0),
                    stop=(dc == EC - 1),
                )
            ob = outp.tile([P, 512], F32, tag="ob")
            nc.vector.tensor_add(
                out=ob, in0=ps, in1=bo_bc[:, fb * 512 : (fb + 1) * 512]
            )
            nc.sync.dma_start(
                out=out[
                    g * N + tt * P : g * N + (tt + 1) * P,
                    fb * 512 : (fb + 1) * 512,
                ],
                in_=ob,
            )

        # ---- software-pipelined program order --------------------------
        load_xt(0)
        load_xt(1)
        for ft in range(EC):
            emit_qk_ftile(0, "q", ft)
        for ft in range(EC):
            emit_qk_ftile(0, "k", ft)
        for fb in range(FB):
            emit_v_fb(0, fb)
        load_wo()
        # group-0 attention interleaved with group-1 q/k proj
        for pr in range(EC):
            emit_attn_pair(0, pr)
            emit_qk_ftile(1, "q", pr)
            emit_qk_ftile(1, "k", pr)
        # batched normalization for group 0 (dense ln/exp, no table thrash)
        for pr in range(EC):
            emit_recip_pair(0, pr)
        for fb in range(FB):
            emit_v_fb(1, fb)
        # group-1 attention interleaved with group-0 out proj
        for pr in range(EC):
            emit_attn_pair(1, pr)
            fb, tt = pr // 4, pr % 4
            emit_outproj_unit(0, fb, tt)
        # batched normalization for group 1, with group-1 out proj chains
        # pipelining behind it
        for pr in range(EC):
            emit_recip_pair(1, pr)
        for fb in range(FB):
            for tt in range(NB):
                emit_outproj_unit(1, fb, tt)
    nc.finalize()
    return nc


def _get_nc():
    if "nc" not in _CACHE:
        _CACHE["nc"] = _build_nc()
    return _CACHE["nc"]


def _make_in_maps(x, Wqkv, bqkv, Wout, bout):
    """Host-side sharding: permute tokens to group-major, pre-transpose x."""
    x = np.asarray(x, dtype=np.float32)
    Wqkv = np.asarray(Wqkv, dtype=np.float32)
    bqkv = np.asarray(bqkv, dtype=np.float32)
    Wout = np.ascontiguousarray(np.asarray(Wout, dtype=np.float16))
    bout = np.ascontiguousarray(np.asarray(bout, dtype=np.float32))

    # group-major token order: x_perm[b, g*N + i] = x[b, i*ST + g]
    x_perm = x.reshape(B, N, ST, E).transpose(0, 2, 1, 3)  # [B, ST, N, E]

    # [E, E] -> [ft, p, c, f] tile-major so each SBUF partition reads big runs
    def tile_qk(w):
        return np.ascontiguousarray(
            w.reshape(EC, P, EC, P).transpose(2, 1, 0, 3).astype(np.float16)
        )

    wq = tile_qk(Wqkv[:, 0:E] * SCALE)
    wk = tile_qk(Wqkv[:, E : 2 * E])
    wv = np.ascontiguousarray(Wqkv[:, 2 * E : 3 * E].astype(np.float16))
    bq = np.ascontiguousarray(bqkv[0:E] * SCALE)
    bk = np.ascontiguousarray(bqkv[E : 2 * E])
    bv = np.ascontiguousarray(bqkv[2 * E : 3 * E])

    vones = np.zeros(H * P, dtype=np.float32)
    for h in range(H):
        off = DH if h % 2 == 0 else 0
        vones[h * P + off : h * P + off + DH] = 1.0
    vones = vones.astype(np.float16)

    in_maps = []
    for c in range(NCORES):
        b = c // (NCORES // B)
        g0 = GPC * (c % (NCORES // B))
        xc = x_perm[b, g0 : g0 + GPC].reshape(TOK, E)  # [1024, E]
        xct = np.ascontiguousarray(xc.T.astype(np.float16))  # [E, 1024]
        in_maps.append(
            {
                "xt": xct,
                "wq": wq,
                "wk": wk,
                "wv": wv,
                "wo": Wout,
                "bq": bq,
                "bk": bk,
                "bv": bv,
                "bo": bout,
                "vones": vones,
            }
        )
    return in_maps


def kernel(x, Wqkv, bqkv, Wout, bout):
    from concourse.bass_utils import run_bass_kernel_spmd

    nc = _get_nc()
    in_maps = _make_in_maps(x, Wqkv, bqkv, Wout, bout)
    trace = bool(int(os.environ.get("KERNEL_TRACE", "0")))
    res = run_bass_kernel_spmd(
        nc, in_maps, core_ids=list(range(NCORES)), trace=trace
    )
    _CACHE["last_result"] = res

    # reassemble: core outputs are [1024 tok, E] in group-major token order
    out = np.empty((B, S, E), dtype=np.float32)
    for b in range(B):
        per_b = [res.results[b * (NCORES // B) + j]["out"] for j in range(NCORES // B)]
        perm = np.concatenate(per_b, axis=0)  # [ST*N, E] group-major
        out[b] = perm.reshape(ST, N, E).transpose(1, 0, 2).reshape(S, E)
    return out



# revision 7
# speedup vs baseline: 1.1413x; 1.1413x over previous
"""Strided (residue-group) attention for Trainium2, SPMD across 8 NeuronCores.

Problem: x[B=2,S=4096,E=1024] -> qkv proj -> per-(batch,head,residue-group)
attention (stride 8 -> 8 groups of n=512 tokens) -> out proj.

Sharding: by (batch, residue-group).  B*stride = 16 group-instances; each of
the 8 cores owns 2 (batch,group) pairs = 1024 tokens and computes their FULL
output rows (it holds all 16 heads for its tokens).  The residue groups are
independent, so there are no cross-device collectives at all; the host
permutes tokens into group-major order on the way in and inverts on the way
out.

v2 design notes (vs the v1 baseline at 327us):
  - ScalarE runs ONLY softmax Exp (one ACT table set, zero table reloads
    after the initial one; v1 thrashed 35 table loads between exp/ln sets).
    The softmax reciprocal is DVE: the PV ones-trick yields replicated
    denominator rows; tiny SBUF->SBUF DMAs gather one row per head into a
    compact [16,512] tile, one nc.vector.reciprocal per group inverts it,
    and a replication DMA broadcasts each row back across the partition
    halves for the normalize multiplies.
  - All weights live resident in SBUF in DMA-friendly layouts (>=2KB per
    partition per descriptor); v1 reloaded wq/wk per (group, ftile).
  - v-proj bias is folded into the out-proj bias host-side
    (o'/den = o/den + bv  =>  bo' = bv @ Wout + bout), so the v projection
    is a pure matmul chain evacuated with one tensor_copy per tile.
  - Score matmuls alternate PE row groups (head pair at array rows 0-63 /
    64-127) so consecutive K=64 matmuls stream concurrently.
  - PV for pair pr is emitted one pair behind its scores so the PE always
    has filler (qk-proj of group 1 / out-proj of group 0) while ACT chews
    the exps.
  - Output is stored fp16 (host upcasts); all activations fp16.
"""

import os

import numpy as np

B, S, E = 2, 4096, 1024
H, ST = 16, 8
DH = E // H  # 64
N = S // ST  # 512 tokens per residue group
NCORES = 8
GPC = (B * ST) // NCORES  # 2 (batch,group) pairs per core
TOK = GPC * N  # 1024 tokens per core
P = 128
EC = E // P  # 8 contraction chunks of 128
NB = N // P  # 4 token chunks of 128 per group
FB = 2  # feature blocks of 512 in E
SCALE = 1.0 / float(np.sqrt(DH))

_CACHE: dict = {}


def _build_nc():
    import concourse.bass as bass
    import concourse.bacc as bacc
    import concourse.tile as tile
    from concourse import mybir

    F32 = mybir.dt.float32
    FP16 = mybir.dt.float16
    ADD = mybir.AluOpType.add
    EXP = mybir.ActivationFunctionType.Exp

    nc = bacc.Bacc()
    xt = nc.declare_dram_parameter("xt", [EC, P, TOK], FP16, isOutput=False)
    wqk = nc.declare_dram_parameter("wqk", [2, EC, P, EC, P], FP16, isOutput=False)
    wv = nc.declare_dram_parameter("wv", [EC, P, E], FP16, isOutput=False)
    wo = nc.declare_dram_parameter("wo", [P, FB, EC, 512], FP16, isOutput=False)
    bqk = nc.declare_dram_parameter("bqk", [P, 2, EC], F32, isOutput=False)
    bo = nc.declare_dram_parameter("bo", [E], F32, isOutput=False)
    out = nc.declare_dram_parameter("out", [TOK, E], FP16, isOutput=True)

    with nc.allow_low_precision(reason="fp16 softmax-denominator reciprocal"), \
        tile.TileContext(nc) as tc, tc.tile_pool(name="const", bufs=1) as const, \
        tc.tile_pool(name="xtp", bufs=EC) as xtp, \
        tc.tile_pool(name="wqkp", bufs=16) as wqkp, \
        tc.tile_pool(name="wvp", bufs=EC) as wvp, \
        tc.tile_pool(name="qkp", bufs=12) as qkp, \
        tc.tile_pool(name="vfp", bufs=8) as vfp, \
        tc.tile_pool(name="expp", bufs=4) as expp, \
        tc.tile_pool(name="osbp", bufs=18) as osbp, \
        tc.tile_pool(name="denp", bufs=2) as denp, \
        tc.tile_pool(name="recp", bufs=6) as recp, \
        tc.tile_pool(name="otp", bufs=16) as otp, \
        tc.tile_pool(name="outp", bufs=4) as outp, \
        tc.tile_pool(name="psmm", bufs=2, space="PSUM") as psmm, \
        tc.tile_pool(name="pssc", bufs=2, space="PSUM") as pssc, \
        tc.tile_pool(name="psop", bufs=2, space="PSUM") as psop:

        # ---- resident weights / constants ------------------------------
        # DMA issue order = need order: wq ft0 first (first chain), then x,
        # then the rest of wq/wk, wv, wo.
        wqk_sb = {}  # (which, ft) -> [128, EC, 128]
        wqk_sb[(0, 0)] = wqkp.tile([P, EC, P], FP16, tag="wqk", name="wq0")
        nc.sync.dma_start(out=wqk_sb[(0, 0)], in_=wqk[0, 0])

        xt_sb = []  # per chunk c: [128, TOK] (g0 cols 0:512, g1 cols 512:1024)
        for c in range(EC):
            t = xtp.tile([P, TOK], FP16, tag="xt", name=f"xt{c}")
            nc.sync.dma_start(out=t[:, 0:N], in_=xt[c, :, 0:N])
            xt_sb.append(t)
        for c in range(EC):
            nc.sync.dma_start(out=xt_sb[c][:, N:TOK], in_=xt[c, :, N:TOK])

        for which in range(2):
            for ft in range(EC):
                if (which, ft) == (0, 0):
                    continue
                t = wqkp.tile([P, EC, P], FP16, tag="wqk", name=f"w{which}_{ft}")
                nc.sync.dma_start(out=t, in_=wqk[which, ft])
                wqk_sb[(which, ft)] = t

        wv_sb = []
        for c in range(EC):
            t = wvp.tile([P, E], FP16, tag="wv", name=f"wv{c}")
            nc.sync.dma_start(out=t, in_=wv[c])
            wv_sb.append(t)

        wo_sb = const.tile([P, FB, EC, 512], FP16)
        nc.sync.dma_start(out=wo_sb, in_=wo[:])

        bqk_sb = const.tile([P, 2, EC], F32)
        nc.sync.dma_start(out=bqk_sb, in_=bqk[:])
        bo_bc = const.tile([P, E], F32)
        nc.gpsimd.dma_start(out=bo_bc, in_=bo[:].partition_broadcast(P))

        # ---- state -----------------------------------------------------
        qts = {0: {}, 1: {}}
        kts = {0: {}, 1: {}}
        vfl = {0: [], 1: []}  # per tt: [128, 18, 64] (blk0/17 = ones)
        exs = {}
        osbs = {0: {}, 1: {}}
        den16 = {}
        rec16 = {}
        ots = {0: {}, 1: {}}
        pv_pending = []  # deferred PV emission (software pipeline lag)

        def emit_qk_chain(g, which, ft, alt=True):
            # alt: alternate psmm/psop for 4-deep chain pipelining (only when
            # the attention PV isn't competing for psop)
            use_op = alt and (ft % 2 == 1)
            ps = (psop if use_op else psmm).tile(
                [P, N], F32, tag="po" if use_op else "mm"
            )
            wt = wqk_sb[(which, ft)]
            for c in range(EC):
                nc.tensor.matmul(
                    ps,
                    lhsT=wt[:, c, :],
                    rhs=xt_sb[c][:, g * N : (g + 1) * N],
                    start=(c == 0),
                    stop=(c == EC - 1),
                )
            t = qkp.tile([P, N], FP16, tag="qt" if which == 0 else "kt")
            nc.vector.tensor_scalar(
                out=t,
                in0=ps,
                scalar1=bqk_sb[:, which, ft : ft + 1],
                scalar2=None,
                op0=ADD,
            )
            (qts if which == 0 else kts)[g][ft] = t

        def emit_v_unit(g, fb, tt):
            if fb == 0 and tt == 0:
                for t2 in range(NB):
                    # [128 k-tok, 16 heads, 128]: head block = [v_h | ones]
                    # (even h) or [ones | v_h] (odd h) so PV yields o rows on
                    # one partition half and denominator rows on the other
                    vt = vfp.tile([P, H, P], FP16, tag="vf")
                    nc.vector.memset(vt[:, 0:H:2, DH:P], 1.0)
                    nc.vector.memset(vt[:, 1:H:2, 0:DH], 1.0)
                    vfl[g].append(vt)
            use_op = tt % 2 == 1
            ps = (psop if use_op else psmm).tile(
                [P, 512], F32, tag="po" if use_op else "mm"
            )
            for c in range(EC):
                nc.tensor.matmul(
                    ps,
                    lhsT=xt_sb[c][:, g * N + tt * P : g * N + (tt + 1) * P],
                    rhs=wv_sb[c][:, fb * 512 : (fb + 1) * 512],
                    start=(c == 0),
                    stop=(c == EC - 1),
                )
            # scatter the 8 heads' v into the interleaved layout with two
            # strided copies (even heads -> block cols 0:64, odd -> 64:128)
            vt = vfl[g][tt]
            psv = ps.rearrange("p (j o) -> p j o", j=NB, o=P)
            h0 = fb * EC
            nc.vector.tensor_copy(
                out=vt[:, h0 : h0 + EC : 2, 0:DH], in_=psv[:, :, 0:DH]
            )
            nc.vector.tensor_copy(
                out=vt[:, h0 + 1 : h0 + EC : 2, DH:P], in_=psv[:, :, DH:P]
            )

        def emit_scores(g, pr):
            he, ho = 2 * pr, 2 * pr + 1
            for h in (he, ho):
                exs[(g, h)] = expp.tile([P, NB, N], FP16, tag="exp", name=f"ex{g}_{h}")
            for half in range(2):
                sce = pssc.tile([P, 2, N], F32, tag="sc")
                sco = pssc.tile([P, 2, N], F32, tag="sc")
                # alternate PE row groups (0-63 / 64-127) so the two heads'
                # K=64 matmuls stream concurrently on the array
                for cc in range(2):
                    c = 2 * half + cc
                    nc.tensor.matmul(
                        sce[:, cc],
                        lhsT=kts[g][pr][0:DH, c * P : (c + 1) * P],
                        rhs=qts[g][pr][0:DH, :],
                        start=True,
                        stop=True,
                    )
                    nc.tensor.matmul(
                        sco[:, cc],
                        lhsT=kts[g][pr][DH:P, c * P : (c + 1) * P],
                        rhs=qts[g][pr][DH:P, :],
                        start=True,
                        stop=True,
                    )
                nc.scalar.activation(
                    out=exs[(g, he)][:, 2 * half : 2 * half + 2], in_=sce, func=EXP
                )
                nc.scalar.activation(
                    out=exs[(g, ho)][:, 2 * half : 2 * half + 2], in_=sco, func=EXP
                )

        def emit_pv(g, pr):
            if (g, 0) not in den16:
                den16[(g, 0)] = denp.tile([H, N], FP16, tag="den", name=f"den{g}")
            for h in (2 * pr, 2 * pr + 1):
                po = psop.tile([P, N], F32, tag="po")
                ex = exs[(g, h)]
                for c in range(NB):
                    nc.tensor.matmul(
                        po,
                        lhsT=vfl[g][c][:, h, :],
                        rhs=ex[:, c, :],
                        start=(c == 0),
                        stop=(c == NB - 1),
                    )
                osb = osbp.tile([P, N], FP16, tag="osb")
                nc.vector.tensor_copy(out=osb, in_=po)
                osbs[g][h] = osb
                dr = DH if h % 2 == 0 else 0
                nc.gpsimd.dma_start(
                    out=den16[(g, 0)][h : h + 1, :], in_=osb[dr : dr + 1, :]
                )

        def flush_pv():
            while pv_pending:
                g, pr = pv_pending.pop(0)
                emit_pv(g, pr)

        def queue_pv(g, pr):
            pv_pending.append((g, pr))

        def emit_recip(g):
            r16 = denp.tile([H, N], FP16, tag="rec", name=f"rec{g}")
            nc.vector.reciprocal(out=r16, in_=den16[(g, 0)])
            rec16[g] = r16

        def emit_norm_pair(g, pr):
            he, ho = 2 * pr, 2 * pr + 1
            ot = otp.tile([P, N], FP16, tag="ot")
            rec2 = recp.tile([P, N], FP16, tag="rec2")
            for h, lo in ((he, 0), (ho, DH)):
                s = rec16[g][h : h + 1, :]
                nc.sync.dma_start(
                    out=rec2[lo : lo + DH, :],
                    in_=bass.AP(
                        tensor=s.tensor,
                        offset=s.offset,
                        ap=[list(s.ap[0]), [0, DH], list(s.ap[1])],
                    ),
                )
                nc.vector.tensor_mul(
                    out=ot[lo : lo + DH, :],
                    in0=osbs[g][h][lo : lo + DH, :],
                    in1=rec2[lo : lo + DH, :],
                )
            ots[g][pr] = ot

        def emit_outproj_unit(g, u):
            fb, tt = u // NB, u % NB
            ps = psmm.tile([P, 512], F32, tag="mm")
            for dc in range(EC):
                nc.tensor.matmul(
                    ps,
                    lhsT=ots[g][dc][:, tt * P : (tt + 1) * P],
                    rhs=wo_sb[:, fb, dc, :],
                    start=(dc == 0),
                    stop=(dc == EC - 1),
                )
            ob = outp.tile([P, 512], FP16, tag="ob")
            nc.vector.tensor_add(
                out=ob, in0=ps, in1=bo_bc[:, fb * 512 : (fb + 1) * 512]
            )
            nc.sync.dma_start(
                out=out[
                    g * N + tt * P : g * N + (tt + 1) * P, fb * 512 : (fb + 1) * 512
                ],
                in_=ob,
            )

        # ---- program order ---------------------------------------------
        # W1: qkv(g0).  The first q chain paces with the xt DMA stream.
        for ft in range(EC):
            emit_qk_chain(0, 0, ft)
        for ft in range(EC):
            emit_qk_chain(0, 1, ft)
        for fb in range(FB):
            for tt in range(NB):
                emit_v_unit(0, fb, tt)

        # W2: attn(g0) with qk(g1) as PE filler while ACT runs the exps.
        # PV for pair pr is emitted one pair late (scores->exp latency).
        for pr in range(EC):
            emit_scores(0, pr)
            emit_qk_chain(1, 0, pr, alt=False)
            flush_pv()
            queue_pv(0, pr)
            emit_qk_chain(1, 1, pr, alt=False)
        flush_pv()

        # W3: v(g1); DVE meanwhile inverts the g0 denominators and
        # normalizes into the g0 ot tiles.  recip(0) sits after the first
        # fb=0 evacuations in DVE program order so it doesn't head-block
        # the v-chain evacuations while its gather DMAs land.
        for tt in range(NB):
            emit_v_unit(1, 0, tt)
        emit_recip(0)
        for tt in range(NB):
            emit_v_unit(1, 1, tt)
            emit_norm_pair(0, 2 * tt)
            emit_norm_pair(0, 2 * tt + 1)

        # W4: attn(g1) with outproj(g0) as PE filler.
        for pr in range(EC):
            emit_scores(1, pr)
            emit_outproj_unit(0, pr)
            flush_pv()
            queue_pv(1, pr)
        flush_pv()

        # W5/W6: invert g1 denominators, normalize, out-proj, store.
        emit_recip(1)
        for pr in range(EC):
            emit_norm_pair(1, pr)
        for u in range(EC):
            emit_outproj_unit(1, u)
    nc.finalize()
    return nc


def _get_nc():
    if "nc" not in _CACHE:
        _CACHE["nc"] = _build_nc()
    return _CACHE["nc"]


def _make_in_maps(x, Wqkv, bqkv, Wout, bout):
    """Host-side sharding: permute tokens to group-major, pre-transpose x,
    pack weights into DMA-friendly resident layouts."""
    x = np.asarray(x, dtype=np.float32)
    Wqkv = np.asarray(Wqkv, dtype=np.float32)
    bqkv = np.asarray(bqkv, dtype=np.float32)
    Wout = np.asarray(Wout, dtype=np.float32)
    bout = np.asarray(bout, dtype=np.float32)

    # group-major token order: x_perm[b, g*N + i] = x[b, i*ST + g]
    x_perm = x.reshape(B, N, ST, E).transpose(0, 2, 1, 3)  # [B, ST, N, E]

    # wqk[which][ft][p][c][j] = W[c*128+p, ft*128+j]  (q pre-scaled)
    def tile_qk(w):
        return np.ascontiguousarray(
            w.reshape(EC, P, EC, P).transpose(2, 1, 0, 3).astype(np.float16)
        )

    wqk = np.stack(
        [tile_qk(Wqkv[:, 0:E] * SCALE), tile_qk(Wqkv[:, E : 2 * E])], axis=0
    )
    # wv[c][p][f] = Wv[c*128+p, f]
    wv = np.ascontiguousarray(
        Wqkv[:, 2 * E : 3 * E].reshape(EC, P, E).astype(np.float16)
    )
    # wo[p][fb][dc][j] = Wout[dc*128+p, fb*512+j]
    wo = np.ascontiguousarray(
        Wout.reshape(EC, P, FB, 512).transpose(1, 2, 0, 3).astype(np.float16)
    )
    # bqk[p][which][ft] = bias[ft*128+p]
    bq = (bqkv[0:E] * SCALE).reshape(EC, P).T
    bk = bqkv[E : 2 * E].reshape(EC, P).T
    bqk = np.ascontiguousarray(np.stack([bq, bk], axis=1).astype(np.float32))
    # v bias folds into the out bias: o'/den = o/den + bv
    bo = np.ascontiguousarray(
        (bqkv[2 * E : 3 * E] @ Wout + bout).astype(np.float32)
    )

    in_maps = []
    for core in range(NCORES):
        b = core // (NCORES // B)
        g0 = GPC * (core % (NCORES // B))
        xc = x_perm[b, g0 : g0 + GPC].reshape(TOK, E)  # [1024, E]
        xct = np.ascontiguousarray(
            xc.T.reshape(EC, P, TOK).astype(np.float16)
        )
        in_maps.append(
            {"xt": xct, "wqk": wqk, "wv": wv, "wo": wo, "bqk": bqk, "bo": bo}
        )
    return in_maps


def kernel(x, Wqkv, bqkv, Wout, bout):
    from concourse.bass_utils import run_bass_kernel_spmd

    nc = _get_nc()
    in_maps = _make_in_maps(x, Wqkv, bqkv, Wout, bout)
    trace = bool(int(os.environ.get("KERNEL_TRACE", "0")))
    res = run_bass_kernel_spmd(
        nc, in_maps, core_ids=list(range(NCORES)), trace=trace
    )
    _CACHE["last_result"] = res

    # reassemble: core outputs are [1024 tok, E] fp16 in group-major order
    out = np.empty((B, S, E), dtype=np.float32)
    for b in range(B):
        per_b = [
            np.asarray(res.results[b * (NCORES // B) + j]["out"], dtype=np.float32)
            for j in range(NCORES // B)
        ]
        perm = np.concatenate(per_b, axis=0)  # [ST*N, E] group-major
        out[b] = perm.reshape(ST, N, E).transpose(1, 0, 2).reshape(S, E)
    return out


# revision 20
# speedup vs baseline: 1.2382x; 1.0849x over previous
"""Strided (residue-group) attention for Trainium2, SPMD across 8 NeuronCores.

Problem: x[B=2,S=4096,E=1024] -> qkv proj -> per-(batch,head,residue-group)
attention (stride 8 -> 8 groups of n=512 tokens) -> out proj.

Sharding: by (batch, residue-group).  B*stride = 16 group-instances; each of
the 8 cores owns 2 (batch,group) pairs = 1024 tokens and computes their FULL
output rows (it holds all 16 heads for its tokens).  The residue groups are
independent, so there are no cross-device collectives at all; the host
permutes tokens into group-major order on the way in and inverts on the way
out.

v2 design notes (vs the v1 baseline at 327us):
  - ScalarE runs ONLY softmax Exp (one ACT table set, zero table reloads
    after the initial one; v1 thrashed 35 table loads between exp/ln sets).
    The softmax reciprocal is DVE: the PV ones-trick yields replicated
    denominator rows; tiny SBUF->SBUF DMAs gather one row per head into a
    compact [16,512] tile, one nc.vector.reciprocal per group inverts it,
    and a replication DMA broadcasts each row back across the partition
    halves for the normalize multiplies.
  - All weights live resident in SBUF in DMA-friendly layouts (>=2KB per
    partition per descriptor); v1 reloaded wq/wk per (group, ftile).
  - v-proj bias is folded into the out-proj bias host-side
    (o'/den = o/den + bv  =>  bo' = bv @ Wout + bout), so the v projection
    is a pure matmul chain evacuated with one tensor_copy per tile.
  - Score matmuls alternate PE row groups (head pair at array rows 0-63 /
    64-127) so consecutive K=64 matmuls stream concurrently.
  - PV for pair pr is emitted one pair behind its scores so the PE always
    has filler (qk-proj of group 1 / out-proj of group 0) while ACT chews
    the exps.
  - Output is stored fp16 (host upcasts); all activations fp16.
"""

import os

import numpy as np

B, S, E = 2, 4096, 1024
H, ST = 16, 8
DH = E // H  # 64
N = S // ST  # 512 tokens per residue group
NCORES = 8
GPC = (B * ST) // NCORES  # 2 (batch,group) pairs per core
TOK = GPC * N  # 1024 tokens per core
P = 128
EC = E // P  # 8 contraction chunks of 128
NB = N // P  # 4 token chunks of 128 per group
FB = 2  # feature blocks of 512 in E
SCALE = 1.0 / float(np.sqrt(DH))

_CACHE: dict = {}


def _build_nc():
    import concourse.bass as bass
    import concourse.bacc as bacc
    import concourse.tile as tile
    from concourse import mybir

    F32 = mybir.dt.float32
    FP16 = mybir.dt.float16
    ADD = mybir.AluOpType.add
    EXP = mybir.ActivationFunctionType.Exp

    nc = bacc.Bacc()
    # layouts chosen for long per-partition contiguous runs (big DMA
    # descriptors) and few dma_start jobs (sequencer trigger cost)
    xt = nc.declare_dram_parameter("xt", [GPC, P, EC, N], FP16, isOutput=False)
    wqk = nc.declare_dram_parameter(
        "wqk", [2, 2, P, 4, EC, P], FP16, isOutput=False
    )  # [which, ft-half, p, ft-lo, c, 128]
    wv = nc.declare_dram_parameter("wv", [P, EC, E], FP16, isOutput=False)
    wo = nc.declare_dram_parameter("wo", [P, FB, EC, 512], FP16, isOutput=False)
    bqk = nc.declare_dram_parameter("bqk", [P, 2, EC], F32, isOutput=False)
    bo = nc.declare_dram_parameter("bo", [E], F32, isOutput=False)
    out = nc.declare_dram_parameter("out", [TOK, E], FP16, isOutput=True)

    with nc.allow_low_precision(reason="fp16 softmax-denominator reciprocal"), \
        tile.TileContext(nc) as tc, tc.tile_pool(name="const", bufs=1) as const, \
        tc.tile_pool(name="xtp", bufs=2) as xtp, \
        tc.tile_pool(name="wqkp", bufs=4) as wqkp, \
        tc.tile_pool(name="qkp", bufs=10) as qkp, \
        tc.tile_pool(name="vfp", bufs=4) as vfp, \
        tc.tile_pool(name="expp", bufs=4) as expp, \
        tc.tile_pool(name="osbp", bufs=18) as osbp, \
        tc.tile_pool(name="denp", bufs=2) as denp, \
        tc.tile_pool(name="recp", bufs=2) as recp, \
        tc.tile_pool(name="otp", bufs=16) as otp, \
        tc.tile_pool(name="outp", bufs=4) as outp, \
        tc.tile_pool(name="psmm", bufs=2, space="PSUM") as psmm, \
        tc.tile_pool(name="pssc", bufs=2, space="PSUM") as pssc, \
        tc.tile_pool(name="psop", bufs=2, space="PSUM") as psop:

        # ---- resident weights / constants ------------------------------
        # Few big DMA jobs, issued in first-need order: wq(ft0-3), x(g0),
        # wq(ft4-7), wk, x(g1), wv, wo.  Each job moves 8-16KB per
        # partition in long contiguous runs.
        wqk_half = {}  # (which, fthalf) -> [128, 4, EC, 128]
        xt_g = []  # per g: [128, EC, 512]

        def load_wqk(which, fh):
            t = wqkp.tile(
                [P, 4, EC, P], FP16, tag="wqk", name=f"w{which}_{fh}"
            )
            nc.sync.dma_start(out=t, in_=wqk[which, fh])
            wqk_half[(which, fh)] = t

        load_wqk(0, 0)
        t = xtp.tile([P, EC, N], FP16, tag="xt", name="xt0")
        nc.sync.dma_start(out=t, in_=xt[0])
        xt_g.append(t)
        load_wqk(0, 1)
        load_wqk(1, 0)
        load_wqk(1, 1)
        t = xtp.tile([P, EC, N], FP16, tag="xt", name="xt1")
        nc.sync.dma_start(out=t, in_=xt[1])
        xt_g.append(t)
        wv_sb = const.tile([P, EC, E], FP16)
        nc.sync.dma_start(out=wv_sb, in_=wv[:])
        wo_sb = const.tile([P, FB, EC, 512], FP16)
        nc.sync.dma_start(out=wo_sb, in_=wo[:])

        bqk_sb = const.tile([P, 2, EC], F32)
        nc.sync.dma_start(out=bqk_sb, in_=bqk[:])
        bo_bc = const.tile([P, E], F32)
        nc.gpsimd.dma_start(out=bo_bc, in_=bo[:].partition_broadcast(P))

        def wqk_tile(which, ft):
            return wqk_half[(which, ft // 4)][:, ft % 4]

        # ---- state -----------------------------------------------------
        qts = {0: {}, 1: {}}
        kts = {0: {}, 1: {}}
        vfl = {0: [], 1: []}  # per tt: [128, 18, 64] (blk0/17 = ones)
        exs = {}
        osbs = {0: {}, 1: {}}
        den16 = {}
        rec16 = {}
        ots = {0: {}, 1: {}}
        pv_pending = []  # deferred PV emission (software pipeline lag)

        def emit_qk_chain(g, which, ft, alt=True):
            # alt: alternate psmm/psop for 4-deep chain pipelining (only when
            # the attention PV isn't competing for psop)
            use_op = alt and (ft % 2 == 1)
            ps = (psop if use_op else psmm).tile(
                [P, N], F32, tag="po" if use_op else "mm"
            )
            wt = wqk_tile(which, ft)
            for c in range(EC):
                nc.tensor.matmul(
                    ps,
                    lhsT=wt[:, c, :],
                    rhs=xt_g[g][:, c, :],
                    start=(c == 0),
                    stop=(c == EC - 1),
                )
            t = qkp.tile([P, N], FP16, tag="qt" if which == 0 else "kt")
            nc.vector.tensor_scalar(
                out=t,
                in0=ps,
                scalar1=bqk_sb[:, which, ft : ft + 1],
                scalar2=None,
                op0=ADD,
            )
            (qts if which == 0 else kts)[g][ft] = t

        def emit_v_unit(g, fb, tt):
            if fb == 0 and tt == 0:
                for t2 in range(NB):
                    # [128 k-tok, 16 heads, 128]: head block = [v_h | ones]
                    # (even h) or [ones | v_h] (odd h) so PV yields o rows on
                    # one partition half and denominator rows on the other
                    vt = vfp.tile([P, H, P], FP16, tag="vf")
                    nc.vector.memset(vt[:, 0:H:2, DH:P], 1.0)
                    nc.vector.memset(vt[:, 1:H:2, 0:DH], 1.0)
                    vfl[g].append(vt)
            use_op = tt % 2 == 1
            ps = (psop if use_op else psmm).tile(
                [P, 512], F32, tag="po" if use_op else "mm"
            )
            for c in range(EC):
                nc.tensor.matmul(
                    ps,
                    lhsT=xt_g[g][:, c, tt * P : (tt + 1) * P],
                    rhs=wv_sb[:, c, fb * 512 : (fb + 1) * 512],
                    start=(c == 0),
                    stop=(c == EC - 1),
                )
            # scatter the 8 heads' v into the interleaved layout with two
            # strided copies (even heads -> block cols 0:64, odd -> 64:128)
            vt = vfl[g][tt]
            psv = ps.rearrange("p (j o) -> p j o", j=NB, o=P)
            h0 = fb * EC
            nc.vector.tensor_copy(
                out=vt[:, h0 : h0 + EC : 2, 0:DH], in_=psv[:, :, 0:DH]
            )
            nc.vector.tensor_copy(
                out=vt[:, h0 + 1 : h0 + EC : 2, DH:P], in_=psv[:, :, DH:P]
            )

        def emit_scores(g, pr):
            he, ho = 2 * pr, 2 * pr + 1
            for h in (he, ho):
                exs[(g, h)] = expp.tile([P, NB, N], FP16, tag="exp", name=f"ex{g}_{h}")
            for half in range(2):
                sce = pssc.tile([P, 2, N], F32, tag="sc")
                sco = pssc.tile([P, 2, N], F32, tag="sc")
                # alternate PE row groups (0-63 / 64-127) so the two heads'
                # K=64 matmuls stream concurrently on the array
                for cc in range(2):
                    c = 2 * half + cc
                    nc.tensor.matmul(
                        sce[:, cc],
                        lhsT=kts[g][pr][0:DH, c * P : (c + 1) * P],
                        rhs=qts[g][pr][0:DH, :],
                        start=True,
                        stop=True,
                    )
                    nc.tensor.matmul(
                        sco[:, cc],
                        lhsT=kts[g][pr][DH:P, c * P : (c + 1) * P],
                        rhs=qts[g][pr][DH:P, :],
                        start=True,
                        stop=True,
                    )
                nc.scalar.activation(
                    out=exs[(g, he)][:, 2 * half : 2 * half + 2], in_=sce, func=EXP
                )
                nc.scalar.activation(
                    out=exs[(g, ho)][:, 2 * half : 2 * half + 2], in_=sco, func=EXP
                )

        def emit_pv(g, pr):
            if (g, 0) not in den16:
                den16[(g, 0)] = denp.tile([H, N], FP16, tag="den", name=f"den{g}")
            for h in (2 * pr, 2 * pr + 1):
                po = psop.tile([P, N], F32, tag="po")
                ex = exs[(g, h)]
                for c in range(NB):
                    nc.tensor.matmul(
                        po,
                        lhsT=vfl[g][c][:, h, :],
                        rhs=ex[:, c, :],
                        start=(c == 0),
                        stop=(c == NB - 1),
                    )
                osb = osbp.tile([P, N], FP16, tag="osb")
                nc.vector.tensor_copy(out=osb, in_=po)
                osbs[g][h] = osb
                dr = DH if h % 2 == 0 else 0
                nc.gpsimd.dma_start(
                    out=den16[(g, 0)][h : h + 1, :], in_=osb[dr : dr + 1, :]
                )

        def flush_pv():
            while pv_pending:
                g, pr = pv_pending.pop(0)
                emit_pv(g, pr)

        def queue_pv(g, pr):
            pv_pending.append((g, pr))

        def emit_recip(g):
            r16 = denp.tile([H, N], FP16, tag="rec", name=f"rec{g}")
            nc.vector.reciprocal(out=r16, in_=den16[(g, 0)])
            # broadcast the pair's two reciprocal rows across the partition
            # halves in one DMA job per pair:
            #   rec2g[0:64, pr, :] = r16[2pr], rec2g[64:128, pr, :] = r16[2pr+1]
            r2 = recp.tile([P, EC, N], FP16, tag="rec2", name=f"rec2_{g}")
            for pr in range(EC):
                s = r16[2 * pr : 2 * pr + 2, :]
                nc.gpsimd.dma_start(
                    out=r2[:, pr, :],
                    in_=bass.AP(
                        tensor=s.tensor,
                        offset=s.offset,
                        ap=[list(s.ap[0]), [0, DH], list(s.ap[-1])],
                    ),
                )
            rec16[g] = r2

        def emit_norm_pair(g, pr):
            he, ho = 2 * pr, 2 * pr + 1
            ot = otp.tile([P, N], FP16, tag="ot")
            r2 = rec16[g]
            for h, lo in ((he, 0), (ho, DH)):
                nc.vector.tensor_mul(
                    out=ot[lo : lo + DH, :],
                    in0=osbs[g][h][lo : lo + DH, :],
                    in1=r2[lo : lo + DH, pr, :],
                )
            ots[g][pr] = ot

        def emit_outproj_unit(g, u):
            fb, tt = u // NB, u % NB
            ps = psmm.tile([P, 512], F32, tag="mm")
            for dc in range(EC):
                nc.tensor.matmul(
                    ps,
                    lhsT=ots[g][dc][:, tt * P : (tt + 1) * P],
                    rhs=wo_sb[:, fb, dc, :],
                    start=(dc == 0),
                    stop=(dc == EC - 1),
                )
            ob = outp.tile([P, 512], FP16, tag="ob")
            nc.vector.tensor_add(
                out=ob, in0=ps, in1=bo_bc[:, fb * 512 : (fb + 1) * 512]
            )
            nc.sync.dma_start(
                out=out[
                    g * N + tt * P : g * N + (tt + 1) * P, fb * 512 : (fb + 1) * 512
                ],
                in_=ob,
            )

        # ---- program order ---------------------------------------------
        # W1: qkv(g0).  The first q chain paces with the xt DMA stream.
        for ft in range(EC):
            emit_qk_chain(0, 0, ft)
        for ft in range(EC):
            emit_qk_chain(0, 1, ft)
        for fb in range(FB):
            for tt in range(NB):
                emit_v_unit(0, fb, tt)

        # W2: attn(g0) with qk(g1) as PE filler while ACT runs the exps.
        # PV for pair pr is emitted one pair late (scores->exp latency).
        for pr in range(EC):
            emit_scores(0, pr)
            emit_qk_chain(1, 0, pr, alt=False)
            flush_pv()
            queue_pv(0, pr)
            emit_qk_chain(1, 1, pr, alt=False)
        flush_pv()

        # W3: v(g1); DVE meanwhile inverts the g0 denominators and
        # normalizes into the g0 ot tiles.  recip(0) sits after the first
        # fb=0 evacuations in DVE program order so it doesn't head-block
        # the v-chain evacuations while its gather DMAs land.
        for tt in range(NB):
            emit_v_unit(1, 0, tt)
        emit_recip(0)
        for tt in range(NB):
            emit_v_unit(1, 1, tt)
            emit_norm_pair(0, 2 * tt)
            emit_norm_pair(0, 2 * tt + 1)

        # W4: attn(g1) with outproj(g0) as PE filler; keep two units in
        # reserve to cover the g1 recip/normalize latency afterwards.
        for pr in range(EC):
            emit_scores(1, pr)
            if pr < 6:
                emit_outproj_unit(0, pr)
            flush_pv()
            queue_pv(1, pr)
        flush_pv()

        # W5/W6: invert g1 denominators, normalize, out-proj, store.
        emit_recip(1)
        emit_outproj_unit(0, 6)
        emit_norm_pair(1, 0)
        emit_norm_pair(1, 1)
        emit_outproj_unit(0, 7)
        for pr in range(2, EC):
            emit_norm_pair(1, pr)
        for u in range(EC):
            emit_outproj_unit(1, u)
    nc.finalize()
    return nc


def _get_nc():
    if "nc" not in _CACHE:
        _CACHE["nc"] = _build_nc()
    return _CACHE["nc"]


def _make_in_maps(x, Wqkv, bqkv, Wout, bout):
    """Host-side sharding: permute tokens to group-major, pre-transpose x,
    pack weights into DMA-friendly resident layouts."""
    x = np.asarray(x, dtype=np.float32)
    Wqkv = np.asarray(Wqkv, dtype=np.float32)
    bqkv = np.asarray(bqkv, dtype=np.float32)
    Wout = np.asarray(Wout, dtype=np.float32)
    bout = np.asarray(bout, dtype=np.float32)

    # group-major token order: x_perm[b, g*N + i] = x[b, i*ST + g]
    x_perm = x.reshape(B, N, ST, E).transpose(0, 2, 1, 3)  # [B, ST, N, E]

    # wqk[which][fh][p][fl][c][j] = W[c*128+p, (fh*4+fl)*128+j]  (q scaled)
    def tile_qk(w):
        return w.reshape(EC, P, 2, 4, P).transpose(2, 1, 3, 0, 4)

    wqk = np.ascontiguousarray(
        np.stack(
            [tile_qk(Wqkv[:, 0:E] * SCALE), tile_qk(Wqkv[:, E : 2 * E])], axis=0
        ).astype(np.float16)
    )
    # wv[p][c][f] = Wv[c*128+p, f]
    wv = np.ascontiguousarray(
        Wqkv[:, 2 * E : 3 * E].reshape(EC, P, E).transpose(1, 0, 2).astype(np.float16)
    )
    # wo[p][fb][dc][j] = Wout[dc*128+p, fb*512+j]
    wo = np.ascontiguousarray(
        Wout.reshape(EC, P, FB, 512).transpose(1, 2, 0, 3).astype(np.float16)
    )
    # bqk[p][which][ft] = bias[ft*128+p]
    bq = (bqkv[0:E] * SCALE).reshape(EC, P).T
    bk = bqkv[E : 2 * E].reshape(EC, P).T
    bqk = np.ascontiguousarray(np.stack([bq, bk], axis=1).astype(np.float32))
    # v bias folds into the out bias: o'/den = o/den + bv
    bo = np.ascontiguousarray(
        (bqkv[2 * E : 3 * E] @ Wout + bout).astype(np.float32)
    )

    in_maps = []
    for core in range(NCORES):
        b = core // (NCORES // B)
        g0 = GPC * (core % (NCORES // B))
        xc = x_perm[b, g0 : g0 + GPC].reshape(TOK, E)  # [1024, E]
        # xt[g][p][c][t] = x[g*N + t, c*128 + p]
        xct = np.ascontiguousarray(
            xc.T.reshape(EC, P, GPC, N).transpose(2, 1, 0, 3).astype(np.float16)
        )
        in_maps.append(
            {"xt": xct, "wqk": wqk, "wv": wv, "wo": wo, "bqk": bqk, "bo": bo}
        )
    return in_maps


def kernel(x, Wqkv, bqkv, Wout, bout):
    from concourse.bass_utils import run_bass_kernel_spmd

    nc = _get_nc()
    in_maps = _make_in_maps(x, Wqkv, bqkv, Wout, bout)
    trace = bool(int(os.environ.get("KERNEL_TRACE", "0")))
    res = run_bass_kernel_spmd(
        nc, in_maps, core_ids=list(range(NCORES)), trace=trace
    )
    _CACHE["last_result"] = res

    # reassemble: core outputs are [1024 tok, E] fp16 in group-major order
    out = np.empty((B, S, E), dtype=np.float32)
    for b in range(B):
        per_b = [
            np.asarray(res.results[b * (NCORES // B) + j]["out"], dtype=np.float32)
            for j in range(NCORES // B)
        ]
        perm = np.concatenate(per_b, axis=0)  # [ST*N, E] group-major
        out[b] = perm.reshape(ST, N, E).transpose(1, 0, 2).reshape(S, E)
    return out
